# revision 26
# baseline (speedup 1.0000x reference)
"""Trainium2 Bass kernel for NeuralCausalDiscovery (28 grouped MLP mechanisms,
BatchNorm (training stats) + exact GELU, batch 32768).

Sharding: expert-parallel over the 28 independent mechanisms. 8 cores x 4
mechanism slots (cores 4-7 carry one duplicated pad slot). Each core processes
the FULL batch for its mechanisms, so BN batch statistics are exact with no
collectives.

Per-core dataflow (all activations in [feature, batch] layout, 2 groups of 2
mechanisms = 128 partitions):
  - Gram: G_aug = [X|1]^T [X|1] accumulated on PE (4 concurrent col-tiles).
    BN1 stats derive algebraically from G (linear-layer + Gram identity), so
    layer-1 BN+GELU is a single fused ACT op straight out of PSUM.
  - L1: h1 = W1eff @ X^T (fp16 matmuls), gelu1 = Gelu(s1*h1+b1adj) PSUM->SBUF.
  - L2 pass A: h2 streamed to PSUM, bn_stats consumes it (stats only).
  - L2 pass B: h2 recomputed, gelu2 fused with BN2 at PSUM read.
  - L3: out = W3 @ a2, [2,512] PSUM tiles DMA'd straight to DRAM.
"""

import numpy as np

N_VARS, HID, BATCH = 28, 64, 32768
EPS = 1e-5
SUB = 512           # matmul N / bn_stats subgroup width
CHUNK = 1024        # psum chunk columns (2 banks)
NSUB_TOT = BATCH // SUB
N_CORES = 8

_CACHE = {}


def _assignments():
    """Per-core list of 4 mechanism slots (cores 4-7 pad with a duplicate)."""
    cores = []
    idx = 0
    for c in range(N_CORES):
        k = 4 if c < 4 else 3
        mechs = list(range(idx, idx + k))
        idx += k
        while len(mechs) < 4:
            mechs.append(mechs[0])
        cores.append(mechs)
    assert idx == N_VARS
    return cores


def _chunks():
    out = []
    c0 = 0
    while c0 < BATCH:
        ln = min(CHUNK, BATCH - c0)
        out.append((c0, ln))
        c0 += ln
    return out


def _build_module():
    import concourse.bass as bass
    import concourse.tile as tile
    from concourse import mybir

    dt = mybir.dt
    f32, f16 = dt.float32, dt.float16
    AF = mybir.ActivationFunctionType
    AX = mybir.AxisListType
    ALU = mybir.AluOpType
    B = BATCH

    nc = bass.Bass(target_bir_lowering=False)

    Xaug = nc.declare_dram_parameter("Xaug", [B, 29], f16, isOutput=False).ap()
    XT16 = nc.declare_dram_parameter("XT16", [28, B], f16, isOutput=False).ap()
    W116 = nc.declare_dram_parameter("W116", [2, 28, 128], f16, isOutput=False).ap()
    W132 = nc.declare_dram_parameter("W132", [2, 28, 128], f32, isOutput=False).ap()
    W1T32 = nc.declare_dram_parameter("W1T32", [2, 128, 28], f32, isOutput=False).ap()
    W216 = nc.declare_dram_parameter("W216", [2, 128, 128], f16, isOutput=False).ap()
    W316 = nc.declare_dram_parameter("W316", [2, 128, 32], f16, isOutput=False).ap()
    GB = nc.declare_dram_parameter("GB", [128, 8], f32, isOutput=False).ap()
    OUT = nc.declare_dram_parameter("OUT", [4, B], f32, isOutput=True).ap()

    Xr4 = Xaug.rearrange("(n c p) d -> n c p d", c=4, p=128)  # [64, 4, 128, 29]

    with tile.TileContext(nc) as tc:
        with (
            tc.tile_pool(name="big", bufs=2) as big,          # a1 buffers
            tc.tile_pool(name="xa", bufs=3) as xap,           # gram input chunks
            tc.tile_pool(name="xt", bufs=5) as xtp,           # XT streaming chunks
            tc.tile_pool(name="a2", bufs=3) as a2p,           # gelu2 output ring
            tc.tile_pool(name="wts", bufs=1) as wts,          # weights + vectors
            tc.tile_pool(name="hps", bufs=2, space="PSUM") as hps,    # h psum chunks
            tc.tile_pool(name="bps", bufs=2, space="PSUM") as bps,    # phase-B h2 psum
            tc.tile_pool(name="ops", bufs=2, space="PSUM") as ops,    # L3/gram/stats psum
        ):
            # ---------------- weight / constant loads ----------------
            epsT = wts.tile([128, 1], f32, tag="eps")
            nc.vector.memset(epsT, EPS)
            gb = wts.tile([128, 8], f32, tag="gb")
            nc.gpsimd.dma_start(out=gb, in_=GB)
            w116 = []
            w132 = []
            w1T32 = []
            w216 = []
            w316 = []
            for g in range(2):
                t = wts.tile([28, 128], f16, tag=f"w116_{g}")
                nc.gpsimd.dma_start(out=t, in_=W116[g])
                w116.append(t)
                t = wts.tile([28, 128], f32, tag=f"w132_{g}")
                nc.gpsimd.dma_start(out=t, in_=W132[g])
                w132.append(t)
                t = wts.tile([128, 28], f32, tag=f"w1T32_{g}")
                nc.gpsimd.dma_start(out=t, in_=W1T32[g])
                w1T32.append(t)
                t = wts.tile([128, 128], f16, tag=f"w216_{g}")
                nc.gpsimd.dma_start(out=t, in_=W216[g])
                w216.append(t)
                t = wts.tile([128, 32], f16, tag=f"w316_{g}")
                nc.gpsimd.dma_start(out=t, in_=W316[g])
                w316.append(t)

            # ---------------- phase 0: Gram of [X|1] ----------------
            # 4 concurrent accumulation chains on PE col-groups 0..3.
            g_ps = ops.tile([128, SUB], f32, tag="op", name="g_ps")
            for half in range(4):
                xa = xap.tile([128, 64, 29], f16, tag="xa")
                eng = nc.sync if half % 2 == 0 else nc.gpsimd
                eng.dma_start(
                    out=xa, in_=Xr4[16 * half : 16 * (half + 1)]
                    .rearrange("n c p d -> p (n c) d")
                )
                for jj in range(64):
                    t = 64 * half + jj
                    j = t % 4
                    nc.tensor.matmul(
                        out=g_ps[32 * j : 32 * j + 28, 0:29],
                        lhsT=xa[:, jj, 0:28],
                        rhs=xa[:, jj, 0:29],
                        start=(t < 4),
                        stop=(t >= 252),
                        tile_position=(0, 32 * j),
                        skip_group_check=True,
                    )
            # reduce the 4 partial grams -> g_sb [28, 29]
            g_sb = wts.tile([28, 29], f32, tag="g_sb")
            nc.vector.tensor_copy(out=g_sb, in_=g_ps[0:28, 0:29])
            for j in range(1, 4):
                nc.vector.tensor_add(
                    out=g_sb, in0=g_sb, in1=g_ps[32 * j : 32 * j + 28, 0:29]
                )

            # ---------------- per-group BN1 stats from Gram ----------------
            s1v, b1v = [], []
            for g in range(2):
                cs = wts.tile([28, 1], f32, tag=f"cs_{g}")
                nc.vector.tensor_scalar_mul(cs, g_sb[:, 28:29], 1.0 / B)
                mean_ps = ops.tile([128, SUB], f32, tag="op", name="mean_ps")
                nc.tensor.matmul(out=mean_ps[:, 0:1], lhsT=w132[g], rhs=cs)
                m1 = wts.tile([128, 1], f32, tag=f"m1_{g}")
                nc.vector.tensor_copy(out=m1, in_=mean_ps[:, 0:1])

                t2_ps = ops.tile([128, SUB], f32, tag="op", name="t2_ps")
                nc.tensor.matmul(out=t2_ps[:, 0:28], lhsT=w132[g], rhs=g_sb[:, 0:28])
                tmp = wts.tile([128, 28], f32, tag=f"tmp28_{g}")
                nc.vector.tensor_mul(out=tmp, in0=t2_ps[:, 0:28], in1=w1T32[g])
                e2 = wts.tile([128, 1], f32, tag=f"e2_{g}")
                nc.vector.tensor_reduce(
                    out=e2, in_=tmp, axis=AX.X, op=ALU.add
                )
                nc.vector.tensor_scalar_mul(e2, e2, 1.0 / B)
                m1sq = wts.tile([128, 1], f32, tag=f"m1sq_{g}")
                nc.vector.tensor_mul(out=m1sq, in0=m1, in1=m1)
                var1 = wts.tile([128, 1], f32, tag=f"var1_{g}")
                nc.vector.tensor_sub(out=var1, in0=e2, in1=m1sq)
                sd1 = wts.tile([128, 1], f32, tag=f"sd1_{g}")
                nc.scalar.activation(sd1, var1, AF.Sqrt, bias=epsT, scale=1.0)
                rinv = wts.tile([128, 1], f32, tag=f"rinv_{g}")
                nc.vector.reciprocal(rinv, sd1)
                s1 = wts.tile([128, 1], f32, tag=f"s1_{g}")
                nc.vector.tensor_mul(out=s1, in0=rinv, in1=gb[:, 4 * g : 4 * g + 1])
                tb = wts.tile([128, 1], f32, tag=f"tb_{g}")
                nc.vector.tensor_mul(out=tb, in0=s1, in1=m1)
                b1 = wts.tile([128, 1], f32, tag=f"b1_{g}")
                nc.vector.tensor_sub(
                    out=b1, in0=gb[:, 4 * g + 1 : 4 * g + 2], in1=tb
                )
                s1v.append(s1)
                b1v.append(b1)

            chunks = _chunks()

            a1t = [
                big.tile([128, B], f16, tag="a1", name="a1_g0"),
                big.tile([128, B], f16, tag="a1", name="a1_g1"),
            ]

            def phase_A(g):
                """L1 + fused BN1-GELU -> a1[g]."""
                XTCH = 2048
                xts = {}

                def load_xt(x0):
                    if x0 not in xts and x0 < B:
                        xt = xtp.tile([28, XTCH], f16, tag="xt", name="xt")
                        nc.gpsimd.dma_start(out=xt, in_=XT16[:, x0 : x0 + XTCH])
                        xts[x0] = xt

                load_xt(0)
                load_xt(XTCH)
                for (c0, ln) in chunks:
                    hp = hps.tile([128, CHUNK], f32, tag="hp")
                    for s in range(ln // SUB):
                        cc = c0 + s * SUB
                        x0 = (cc // XTCH) * XTCH
                        load_xt(x0 + XTCH)
                        xt = xts[x0]
                        xo = cc % XTCH
                        nc.tensor.matmul(
                            out=hp[:, s * SUB : (s + 1) * SUB],
                            lhsT=w116[g],
                            rhs=xt[:, xo : xo + SUB],
                        )
                    nc.scalar.activation(
                        a1t[g][:, c0 : c0 + ln],
                        hp[:, 0:ln],
                        AF.Gelu,
                        bias=b1v[g],
                        scale=s1v[g],
                    )

            def phase_B(g, st):
                """L2 stats pass: h2 -> PSUM -> bn_stats, h2 discarded."""
                for sub in range(NSUB_TOT):
                    hp = bps.tile([128, SUB], f32, tag="hb", name="hb")
                    nc.tensor.matmul(
                        out=hp,
                        lhsT=w216[g],
                        rhs=a1t[g][:, sub * SUB : (sub + 1) * SUB],
                    )
                    nc.vector.bn_stats(out=st[:, sub, :], in_=hp)

            def stats2(g, st):
                mv = wts.tile([128, 2], f32, tag=f"mv_{g}")
                nc.vector.bn_aggr(out=mv, in_=st)
                sd2 = wts.tile([128, 1], f32, tag=f"sd2_{g}")
                nc.scalar.activation(sd2, mv[:, 1:2], AF.Sqrt, bias=epsT, scale=1.0)
                rinv2 = wts.tile([128, 1], f32, tag=f"rinv2_{g}")
                nc.vector.reciprocal(rinv2, sd2)
                s2 = wts.tile([128, 1], f32, tag=f"s2_{g}")
                nc.vector.tensor_mul(
                    out=s2, in0=rinv2, in1=gb[:, 4 * g + 2 : 4 * g + 3]
                )
                tb2 = wts.tile([128, 1], f32, tag=f"tb2_{g}")
                nc.vector.tensor_mul(out=tb2, in0=s2, in1=mv[:, 0:1])
                b2 = wts.tile([128, 1], f32, tag=f"b2_{g}")
                nc.vector.tensor_sub(
                    out=b2, in0=gb[:, 4 * g + 3 : 4 * g + 4], in1=tb2
                )
                return s2, b2

            def phase_C(g, s2, b2):
                """L2 recompute + fused BN2-GELU + L3 (4 sub-chunks packed per
                PSUM bank at partition offsets 0/32/64/96) + copy + DMA out."""
                sub_global = 0
                op = None
                pend = []
                for (c0, ln) in chunks:
                    hp = hps.tile([128, CHUNK], f32, tag="hp")
                    ns = ln // SUB
                    for s in range(ns):
                        nc.tensor.matmul(
                            out=hp[:, s * SUB : (s + 1) * SUB],
                            lhsT=w216[g],
                            rhs=a1t[g][:, c0 + s * SUB : c0 + (s + 1) * SUB],
                        )
                    a2 = a2p.tile([128, CHUNK], f16, tag="a2")
                    nc.scalar.activation(
                        a2[:, 0:ln], hp[:, 0:ln], AF.Gelu, bias=b2, scale=s2
                    )
                    for s in range(ns):
                        q = sub_global % 4
                        j = (sub_global // 4) % 4
                        if q == 0:
                            op = ops.tile([128, SUB], f32, tag="op", name="op")
                        if sub_global % 16 == 0:
                            stg = a2p.tile([128, 4 * SUB], f32, tag="stg",
                                           name="stg")
                            base8k = c0 + s * SUB
                        nc.tensor.matmul(
                            out=op[32 * q : 32 * q + 32, :],
                            lhsT=w316[g],
                            rhs=a2[:, s * SUB : (s + 1) * SUB],
                            tile_position=(0, 32 * q),
                            skip_group_check=True,
                        )
                        if q == 3:
                            nc.vector.tensor_copy(
                                out=stg[:, j * SUB : (j + 1) * SUB], in_=op
                            )
                        if sub_global % 16 == 15:
                            row16 = OUT[2 * g : 2 * g + 2,
                                        base8k : base8k + 16 * SUB]
                            for qq in range(4):
                                for r in range(2):
                                    out_ap = row16[r : r + 1].rearrange(
                                        "a (jj q f) -> a jj q f", q=4, f=SUB
                                    )[:, :, qq, :]
                                    in_ap = stg[
                                        32 * qq + r : 32 * qq + r + 1, :
                                    ].rearrange("a (jj f) -> a jj f", f=SUB)
                                    nc.sync.dma_start(out=out_ap, in_=in_ap)
                        sub_global += 1

            st0 = wts.tile([128, NSUB_TOT, 6], f32, tag="st_0")
            st1 = wts.tile([128, NSUB_TOT, 6], f32, tag="st_1")

            # pipelined schedule: B(g) overlaps A(g+1); B(1) overlaps C(0)
            phase_A(0)
            phase_B(0, st0)
            s2_0, b2_0 = stats2(0, st0)
            phase_A(1)
            phase_B(1, st1)
            phase_C(0, s2_0, b2_0)
            s2_1, b2_1 = stats2(1, st1)
            phase_C(1, s2_1, b2_1)

    _split_multi_waits(nc, mybir)
    return nc


def _split_multi_waits(nc, mybir):
    """This walrus build accepts only one sync-wait per instruction; hoist
    extra waits onto standalone NoOps on the same engine stream."""
    for fn in nc.m.functions:
        for blk in fn.blocks:
            insts = list(blk.instructions)
            out = []
            nsplit = 0
            for inst in insts:
                si = inst.sync_info
                if si is not None and si.on_wait and len(si.on_wait) > 1:
                    waits = list(si.on_wait)
                    for w in waits[:-1]:
                        nop = mybir.InstNoOp(
                            name=nc.get_next_instruction_name(),
                            engine=inst.engine,
                            ins=[],
                            outs=[],
                            sync_info=mybir.SyncInfo(on_wait=[w], on_update=[]),
                            bass_nofuse=True,
                        )
                        nc.register_instruction(nop)
                        out.append(nop)
                        nsplit += 1
                    inst.sync_info = mybir.SyncInfo(
                        on_wait=[waits[-1]], on_update=list(si.on_update)
                    )
                out.append(inst)
            if nsplit:
                li = blk.instructions
                li.clear()
                for i in out:
                    blk.add_instruction(i)


def _get_nc():
    if "nc" not in _CACHE:
        _CACHE["nc"] = _build_module()
    return _CACHE["nc"]


def _host_prep(X, W_logits, W1, b1, gamma1, beta1, W2, b2, gamma2, beta2, W3, b3):
    """Build per-core input maps. Weight-only preprocessing on host."""
    X = np.asarray(X, np.float32)
    W_logits = np.asarray(W_logits, np.float32)
    W1 = np.asarray(W1, np.float32)
    W2 = np.asarray(W2, np.float32)
    W3 = np.asarray(W3, np.float32)
    gamma1 = np.asarray(gamma1, np.float32)
    beta1 = np.asarray(beta1, np.float32)
    gamma2 = np.asarray(gamma2, np.float32)
    beta2 = np.asarray(beta2, np.float32)

    W = (1.0 / (1.0 + np.exp(-W_logits))) * (1.0 - np.eye(N_VARS, dtype=np.float32))
    W1eff = W1 * W[:, None, :]  # [n, h, n]

    Xaug = np.ones((BATCH, 29), np.float16)
    Xaug[:, 0:28] = X.astype(np.float16)
    XT16 = np.ascontiguousarray(X.T).astype(np.float16)

    assigns = _assignments()
    in_maps = []
    for c in range(N_CORES):
        mechs = assigns[c]
        W1g = np.zeros((2, 28, 128), np.float32)
        W2g = np.zeros((2, 128, 128), np.float32)
        W3g = np.zeros((2, 128, 2), np.float32)
        GBg = np.zeros((128, 8), np.float32)
        for g in range(2):
            m0, m1 = mechs[2 * g], mechs[2 * g + 1]
            W1g[g, :, 0:64] = W1eff[m0].T
            W1g[g, :, 64:128] = W1eff[m1].T
            W2g[g, 0:64, 0:64] = W2[m0].T
            W2g[g, 64:128, 64:128] = W2[m1].T
            W3g[g, 0:64, 0] = W3[m0, 0, :]
            W3g[g, 64:128, 1] = W3[m1, 0, :]
            GBg[:, 4 * g + 0] = np.concatenate([gamma1[m0], gamma1[m1]])
            GBg[:, 4 * g + 1] = np.concatenate([beta1[m0], beta1[m1]])
            GBg[:, 4 * g + 2] = np.concatenate([gamma2[m0], gamma2[m1]])
            GBg[:, 4 * g + 3] = np.concatenate([beta2[m0], beta2[m1]])
        W1gT = np.ascontiguousarray(np.transpose(W1g, (0, 2, 1)))
        in_maps.append(
            {
                "Xaug": Xaug,
                "XT16": XT16,
                "W116": W1g.astype(np.float16),
                "W132": W1g,
                "W1T32": W1gT,
                "W216": W2g.astype(np.float16),
                "W316": np.tile(W3g, (1, 1, 16)).astype(np.float16),
                "GB": GBg,
            }
        )
    return in_maps, assigns, W


def _assemble(results, assigns, W, b3):
    b3 = np.asarray(b3, np.float32)
    X_hat = np.zeros((BATCH, N_VARS), np.float32)
    done = set()
    for c in range(N_CORES):
        out = results[c]["OUT"]  # [4, B]
        for slot in range(4):
            mech = assigns[c][slot]
            if mech in done:
                continue
            done.add(mech)
            X_hat[:, mech] = out[slot, :] + b3[mech, 0]
    assert len(done) == N_VARS
    return X_hat, W


def run(trace=False, **inputs):
    from concourse.bass_utils import run_bass_kernel_spmd

    in_maps, assigns, W = _host_prep(**inputs)
    nc = _get_nc()
    res = run_bass_kernel_spmd(nc, in_maps, list(range(N_CORES)), trace=trace)
    X_hat, W = _assemble(res.results, assigns, W, inputs["b3"])
    return (X_hat, W), res


def kernel(**inputs):
    (X_hat, W), _ = run(trace=False, **inputs)
    return (X_hat, W)


# revision 29
# speedup vs baseline: 1.0219x; 1.0219x over previous
"""Trainium2 Bass kernel for NeuralCausalDiscovery (28 grouped MLP mechanisms,
BatchNorm (training stats) + exact GELU, batch 32768).

Sharding: expert-parallel over the 28 independent mechanisms. 8 cores x 4
mechanism slots (cores 4-7 carry one duplicated pad slot). Each core processes
the FULL batch for its mechanisms, so BN batch statistics are exact with no
collectives.

Per-core dataflow (all activations in [feature, batch] layout, 2 groups of 2
mechanisms = 128 partitions):
  - Gram: G_aug = [X|1]^T [X|1] accumulated on PE (4 concurrent col-tiles).
    BN1 stats derive algebraically from G (linear-layer + Gram identity), so
    layer-1 BN+GELU is a single fused ACT op straight out of PSUM.
  - L1: h1 = W1eff @ X^T (fp16 matmuls), gelu1 = Gelu(s1*h1+b1adj) PSUM->SBUF.
  - L2 pass A: h2 streamed to PSUM, bn_stats consumes it (stats only).
  - L2 pass B: h2 recomputed, gelu2 fused with BN2 at PSUM read.
  - L3: out = W3 @ a2, [2,512] PSUM tiles DMA'd straight to DRAM.
"""

import sys

import numpy as np

if "/opt/trn_rl_repo" not in sys.path:
    sys.path.insert(0, "/opt/trn_rl_repo")

N_VARS, HID, BATCH = 28, 64, 32768
EPS = 1e-5
SUB = 512           # matmul N / bn_stats subgroup width
CHUNK = 1024        # psum chunk columns (2 banks)
NSUB_TOT = BATCH // SUB
N_CORES = 8

_CACHE = {}


def _assignments():
    """Per-core list of 4 mechanism slots (cores 4-7 pad with a duplicate)."""
    cores = []
    idx = 0
    for c in range(N_CORES):
        k = 4 if c < 4 else 3
        mechs = list(range(idx, idx + k))
        idx += k
        while len(mechs) < 4:
            mechs.append(mechs[0])
        cores.append(mechs)
    assert idx == N_VARS
    return cores


def _chunks():
    out = []
    c0 = 0
    while c0 < BATCH:
        ln = min(CHUNK, BATCH - c0)
        out.append((c0, ln))
        c0 += ln
    return out


def _build_module():
    import concourse.bass as bass
    import concourse.tile as tile
    from concourse import mybir

    dt = mybir.dt
    f32, f16 = dt.float32, dt.float16
    AF = mybir.ActivationFunctionType
    AX = mybir.AxisListType
    ALU = mybir.AluOpType
    B = BATCH

    nc = bass.Bass(target_bir_lowering=False)

    Xaug = nc.declare_dram_parameter("Xaug", [B, 29], f16, isOutput=False).ap()
    XT16 = nc.declare_dram_parameter("XT16", [28, B], f16, isOutput=False).ap()
    W116 = nc.declare_dram_parameter("W116", [2, 28, 128], f16, isOutput=False).ap()
    W132 = nc.declare_dram_parameter("W132", [2, 28, 128], f32, isOutput=False).ap()
    W1T32 = nc.declare_dram_parameter("W1T32", [2, 128, 28], f32, isOutput=False).ap()
    W216 = nc.declare_dram_parameter("W216", [2, 128, 128], f16, isOutput=False).ap()
    W316 = nc.declare_dram_parameter("W316", [2, 128, 32], f16, isOutput=False).ap()
    GB = nc.declare_dram_parameter("GB", [128, 8], f32, isOutput=False).ap()
    OUT = nc.declare_dram_parameter("OUT", [4, B], f32, isOutput=True).ap()

    Xr4 = Xaug.rearrange("(n c p) d -> n c p d", c=4, p=128)  # [64, 4, 128, 29]

    with tile.TileContext(nc) as tc:
        with (
            tc.tile_pool(name="big", bufs=2) as big,          # a1 buffers
            tc.tile_pool(name="xa", bufs=4) as xap,           # gram input chunks
            tc.tile_pool(name="xt", bufs=5) as xtp,           # XT streaming chunks
            tc.tile_pool(name="a2", bufs=3) as a2p,           # gelu2 output ring
            tc.tile_pool(name="wts", bufs=1) as wts,          # weights + vectors
            tc.tile_pool(name="hps", bufs=2, space="PSUM") as hps,    # h psum chunks
            tc.tile_pool(name="bps", bufs=2, space="PSUM") as bps,    # phase-B h2 psum
            tc.tile_pool(name="ops", bufs=2, space="PSUM") as ops,    # L3/gram/stats psum
        ):
            # ---------------- weight / constant loads ----------------
            epsT = wts.tile([128, 1], f32, tag="eps")
            nc.vector.memset(epsT, EPS)
            gb = wts.tile([128, 8], f32, tag="gb")
            nc.gpsimd.dma_start(out=gb, in_=GB)
            w116 = []
            w132 = []
            w1T32 = []
            w216 = []
            w316 = []
            for g in range(2):
                t = wts.tile([28, 128], f16, tag=f"w116_{g}")
                nc.gpsimd.dma_start(out=t, in_=W116[g])
                w116.append(t)
                t = wts.tile([28, 128], f32, tag=f"w132_{g}")
                nc.gpsimd.dma_start(out=t, in_=W132[g])
                w132.append(t)
                t = wts.tile([128, 28], f32, tag=f"w1T32_{g}")
                nc.gpsimd.dma_start(out=t, in_=W1T32[g])
                w1T32.append(t)
                t = wts.tile([128, 128], f16, tag=f"w216_{g}")
                nc.gpsimd.dma_start(out=t, in_=W216[g])
                w216.append(t)
                t = wts.tile([128, 32], f16, tag=f"w316_{g}")
                nc.gpsimd.dma_start(out=t, in_=W316[g])
                w316.append(t)

            # ---------------- phase 0: Gram of [X|1] ----------------
            # 4 concurrent accumulation chains on PE col-groups 0..3.
            g_ps = ops.tile([128, SUB], f32, tag="op", name="g_ps")
            for half in range(8):
                xa = xap.tile([128, 32, 29], f16, tag="xa")
                eng = nc.sync if half % 2 == 0 else nc.gpsimd
                eng.dma_start(
                    out=xa, in_=Xr4[8 * half : 8 * (half + 1)]
                    .rearrange("n c p d -> p (n c) d")
                )
                for jj in range(32):
                    t = 32 * half + jj
                    j = t % 4
                    nc.tensor.matmul(
                        out=g_ps[32 * j : 32 * j + 28, 0:29],
                        lhsT=xa[:, jj, 0:28],
                        rhs=xa[:, jj, 0:29],
                        start=(t < 4),
                        stop=(t >= 252),
                        tile_position=(0, 32 * j),
                        skip_group_check=True,
                    )
            # reduce the 4 partial grams -> g_sb [28, 29]
            g_sb = wts.tile([28, 29], f32, tag="g_sb")
            nc.vector.tensor_copy(out=g_sb, in_=g_ps[0:28, 0:29])
            for j in range(1, 4):
                nc.vector.tensor_add(
                    out=g_sb, in0=g_sb, in1=g_ps[32 * j : 32 * j + 28, 0:29]
                )

            # ---------------- per-group BN1 stats from Gram ----------------
            s1v, b1v = [], []
            for g in range(2):
                cs = wts.tile([28, 1], f32, tag=f"cs_{g}")
                nc.vector.tensor_scalar_mul(cs, g_sb[:, 28:29], 1.0 / B)
                mean_ps = ops.tile([128, SUB], f32, tag="op", name="mean_ps")
                nc.tensor.matmul(out=mean_ps[:, 0:1], lhsT=w132[g], rhs=cs)
                m1 = wts.tile([128, 1], f32, tag=f"m1_{g}")
                nc.vector.tensor_copy(out=m1, in_=mean_ps[:, 0:1])

                t2_ps = ops.tile([128, SUB], f32, tag="op", name="t2_ps")
                nc.tensor.matmul(out=t2_ps[:, 0:28], lhsT=w132[g], rhs=g_sb[:, 0:28])
                tmp = wts.tile([128, 28], f32, tag=f"tmp28_{g}")
                nc.vector.tensor_mul(out=tmp, in0=t2_ps[:, 0:28], in1=w1T32[g])
                e2 = wts.tile([128, 1], f32, tag=f"e2_{g}")
                nc.vector.tensor_reduce(
                    out=e2, in_=tmp, axis=AX.X, op=ALU.add
                )
                nc.vector.tensor_scalar_mul(e2, e2, 1.0 / B)
                m1sq = wts.tile([128, 1], f32, tag=f"m1sq_{g}")
                nc.vector.tensor_mul(out=m1sq, in0=m1, in1=m1)
                var1 = wts.tile([128, 1], f32, tag=f"var1_{g}")
                nc.vector.tensor_sub(out=var1, in0=e2, in1=m1sq)
                sd1 = wts.tile([128, 1], f32, tag=f"sd1_{g}")
                nc.scalar.activation(sd1, var1, AF.Sqrt, bias=epsT, scale=1.0)
                rinv = wts.tile([128, 1], f32, tag=f"rinv_{g}")
                nc.vector.reciprocal(rinv, sd1)
                s1 = wts.tile([128, 1], f32, tag=f"s1_{g}")
                nc.vector.tensor_mul(out=s1, in0=rinv, in1=gb[:, 4 * g : 4 * g + 1])
                tb = wts.tile([128, 1], f32, tag=f"tb_{g}")
                nc.vector.tensor_mul(out=tb, in0=s1, in1=m1)
                b1 = wts.tile([128, 1], f32, tag=f"b1_{g}")
                nc.vector.tensor_sub(
                    out=b1, in0=gb[:, 4 * g + 1 : 4 * g + 2], in1=tb
                )
                s1v.append(s1)
                b1v.append(b1)

            chunks = _chunks()

            a1t = [
                big.tile([128, B], f16, tag="a1", name="a1_g0"),
                big.tile([128, B], f16, tag="a1", name="a1_g1"),
            ]

            def phase_A(g):
                """L1 + fused BN1-GELU -> a1[g]."""
                XTCH = 2048
                xts = {}

                def load_xt(x0):
                    if x0 not in xts and x0 < B:
                        xt = xtp.tile([28, XTCH], f16, tag="xt", name="xt")
                        nc.gpsimd.dma_start(out=xt, in_=XT16[:, x0 : x0 + XTCH])
                        xts[x0] = xt

                load_xt(0)
                load_xt(XTCH)
                for (c0, ln) in chunks:
                    hp = hps.tile([128, CHUNK], f32, tag="hp")
                    for s in range(ln // SUB):
                        cc = c0 + s * SUB
                        x0 = (cc // XTCH) * XTCH
                        load_xt(x0 + XTCH)
                        xt = xts[x0]
                        xo = cc % XTCH
                        nc.tensor.matmul(
                            out=hp[:, s * SUB : (s + 1) * SUB],
                            lhsT=w116[g],
                            rhs=xt[:, xo : xo + SUB],
                        )
                    nc.scalar.activation(
                        a1t[g][:, c0 : c0 + ln],
                        hp[:, 0:ln],
                        AF.Gelu,
                        bias=b1v[g],
                        scale=s1v[g],
                    )

            def phase_B(g, st):
                """L2 stats pass: h2 -> PSUM -> bn_stats, h2 discarded."""
                for sub in range(NSUB_TOT):
                    hp = bps.tile([128, SUB], f32, tag="hb", name="hb")
                    nc.tensor.matmul(
                        out=hp,
                        lhsT=w216[g],
                        rhs=a1t[g][:, sub * SUB : (sub + 1) * SUB],
                    )
                    nc.vector.bn_stats(out=st[:, sub, :], in_=hp)

            def stats2(g, st):
                mv = wts.tile([128, 2], f32, tag=f"mv_{g}")
                nc.vector.bn_aggr(out=mv, in_=st)
                sd2 = wts.tile([128, 1], f32, tag=f"sd2_{g}")
                nc.scalar.activation(sd2, mv[:, 1:2], AF.Sqrt, bias=epsT, scale=1.0)
                rinv2 = wts.tile([128, 1], f32, tag=f"rinv2_{g}")
                nc.vector.reciprocal(rinv2, sd2)
                s2 = wts.tile([128, 1], f32, tag=f"s2_{g}")
                nc.vector.tensor_mul(
                    out=s2, in0=rinv2, in1=gb[:, 4 * g + 2 : 4 * g + 3]
                )
                tb2 = wts.tile([128, 1], f32, tag=f"tb2_{g}")
                nc.vector.tensor_mul(out=tb2, in0=s2, in1=mv[:, 0:1])
                b2 = wts.tile([128, 1], f32, tag=f"b2_{g}")
                nc.vector.tensor_sub(
                    out=b2, in0=gb[:, 4 * g + 3 : 4 * g + 4], in1=tb2
                )
                return s2, b2

            def phase_C(g, s2, b2):
                """L2 recompute + fused BN2-GELU + L3 (4 sub-chunks packed per
                PSUM bank at partition offsets 0/32/64/96) + copy + DMA out."""
                sub_global = 0
                op = None
                pend = []
                for (c0, ln) in chunks:
                    hp = hps.tile([128, CHUNK], f32, tag="hp")
                    ns = ln // SUB
                    for s in range(ns):
                        nc.tensor.matmul(
                            out=hp[:, s * SUB : (s + 1) * SUB],
                            lhsT=w216[g],
                            rhs=a1t[g][:, c0 + s * SUB : c0 + (s + 1) * SUB],
                        )
                    a2 = a2p.tile([128, CHUNK], f16, tag="a2")
                    nc.scalar.activation(
                        a2[:, 0:ln], hp[:, 0:ln], AF.Gelu, bias=b2, scale=s2
                    )
                    for s in range(ns):
                        q = sub_global % 4
                        j = (sub_global // 4) % 4
                        if q == 0:
                            op = ops.tile([128, SUB], f32, tag="op", name="op")
                        if sub_global % 16 == 0:
                            stg = a2p.tile([128, 4 * SUB], f32, tag="stg",
                                           name="stg")
                            base8k = c0 + s * SUB
                        nc.tensor.matmul(
                            out=op[32 * q : 32 * q + 32, :],
                            lhsT=w316[g],
                            rhs=a2[:, s * SUB : (s + 1) * SUB],
                            tile_position=(0, 32 * q),
                            skip_group_check=True,
                        )
                        if q == 3:
                            nc.vector.tensor_copy(
                                out=stg[:, j * SUB : (j + 1) * SUB], in_=op
                            )
                        if sub_global % 16 == 15:
                            row16 = OUT[2 * g : 2 * g + 2,
                                        base8k : base8k + 16 * SUB]
                            for qq in range(4):
                                for r in range(2):
                                    out_ap = row16[r : r + 1].rearrange(
                                        "a (jj q f) -> a jj q f", q=4, f=SUB
                                    )[:, :, qq, :]
                                    in_ap = stg[
                                        32 * qq + r : 32 * qq + r + 1, :
                                    ].rearrange("a (jj f) -> a jj f", f=SUB)
                                    nc.sync.dma_start(out=out_ap, in_=in_ap)
                        sub_global += 1

            st0 = wts.tile([128, NSUB_TOT, 6], f32, tag="st_0")
            st1 = wts.tile([128, NSUB_TOT, 6], f32, tag="st_1")

            # pipelined schedule: B(g) overlaps A(g+1); B(1) overlaps C(0)
            phase_A(0)
            phase_B(0, st0)
            s2_0, b2_0 = stats2(0, st0)
            phase_A(1)
            phase_B(1, st1)
            phase_C(0, s2_0, b2_0)
            s2_1, b2_1 = stats2(1, st1)
            phase_C(1, s2_1, b2_1)

    _split_multi_waits(nc, mybir)
    return nc


def _split_multi_waits(nc, mybir):
    """This walrus build accepts only one sync-wait per instruction; hoist
    extra waits onto standalone NoOps on the same engine stream."""
    for fn in nc.m.functions:
        for blk in fn.blocks:
            insts = list(blk.instructions)
            out = []
            nsplit = 0
            for inst in insts:
                si = inst.sync_info
                if si is not None and si.on_wait and len(si.on_wait) > 1:
                    waits = list(si.on_wait)
                    for w in waits[:-1]:
                        nop = mybir.InstNoOp(
                            name=nc.get_next_instruction_name(),
                            engine=inst.engine,
                            ins=[],
                            outs=[],
                            sync_info=mybir.SyncInfo(on_wait=[w], on_update=[]),
                            bass_nofuse=True,
                        )
                        nc.register_instruction(nop)
                        out.append(nop)
                        nsplit += 1
                    inst.sync_info = mybir.SyncInfo(
                        on_wait=[waits[-1]], on_update=list(si.on_update)
                    )
                out.append(inst)
            if nsplit:
                li = blk.instructions
                li.clear()
                for i in out:
                    blk.add_instruction(i)


def _get_nc():
    if "nc" not in _CACHE:
        _CACHE["nc"] = _build_module()
    return _CACHE["nc"]


def _host_prep(X, W_logits, W1, b1, gamma1, beta1, W2, b2, gamma2, beta2, W3, b3):
    """Build per-core input maps. Weight-only preprocessing on host."""
    X = np.asarray(X, np.float32)
    W_logits = np.asarray(W_logits, np.float32)
    W1 = np.asarray(W1, np.float32)
    W2 = np.asarray(W2, np.float32)
    W3 = np.asarray(W3, np.float32)
    gamma1 = np.asarray(gamma1, np.float32)
    beta1 = np.asarray(beta1, np.float32)
    gamma2 = np.asarray(gamma2, np.float32)
    beta2 = np.asarray(beta2, np.float32)

    W = (1.0 / (1.0 + np.exp(-W_logits))) * (1.0 - np.eye(N_VARS, dtype=np.float32))
    W1eff = W1 * W[:, None, :]  # [n, h, n]

    Xaug = np.ones((BATCH, 29), np.float16)
    Xaug[:, 0:28] = X.astype(np.float16)
    XT16 = np.ascontiguousarray(X.T).astype(np.float16)

    assigns = _assignments()
    in_maps = []
    for c in range(N_CORES):
        mechs = assigns[c]
        W1g = np.zeros((2, 28, 128), np.float32)
        W2g = np.zeros((2, 128, 128), np.float32)
        W3g = np.zeros((2, 128, 2), np.float32)
        GBg = np.zeros((128, 8), np.float32)
        for g in range(2):
            m0, m1 = mechs[2 * g], mechs[2 * g + 1]
            W1g[g, :, 0:64] = W1eff[m0].T
            W1g[g, :, 64:128] = W1eff[m1].T
            W2g[g, 0:64, 0:64] = W2[m0].T
            W2g[g, 64:128, 64:128] = W2[m1].T
            W3g[g, 0:64, 0] = W3[m0, 0, :]
            W3g[g, 64:128, 1] = W3[m1, 0, :]
            GBg[:, 4 * g + 0] = np.concatenate([gamma1[m0], gamma1[m1]])
            GBg[:, 4 * g + 1] = np.concatenate([beta1[m0], beta1[m1]])
            GBg[:, 4 * g + 2] = np.concatenate([gamma2[m0], gamma2[m1]])
            GBg[:, 4 * g + 3] = np.concatenate([beta2[m0], beta2[m1]])
        W1gT = np.ascontiguousarray(np.transpose(W1g, (0, 2, 1)))
        in_maps.append(
            {
                "Xaug": Xaug,
                "XT16": XT16,
                "W116": W1g.astype(np.float16),
                "W132": W1g,
                "W1T32": W1gT,
                "W216": W2g.astype(np.float16),
                "W316": np.tile(W3g, (1, 1, 16)).astype(np.float16),
                "GB": GBg,
            }
        )
    return in_maps, assigns, W


def _assemble(results, assigns, W, b3):
    b3 = np.asarray(b3, np.float32)
    X_hat = np.zeros((BATCH, N_VARS), np.float32)
    done = set()
    for c in range(N_CORES):
        out = results[c]["OUT"]  # [4, B]
        for slot in range(4):
            mech = assigns[c][slot]
            if mech in done:
                continue
            done.add(mech)
            X_hat[:, mech] = out[slot, :] + b3[mech, 0]
    assert len(done) == N_VARS
    return X_hat, W


def run(trace=False, **inputs):
    from concourse.bass_utils import run_bass_kernel_spmd

    in_maps, assigns, W = _host_prep(**inputs)
    nc = _get_nc()
    res = run_bass_kernel_spmd(nc, in_maps, list(range(N_CORES)), trace=trace)
    X_hat, W = _assemble(res.results, assigns, W, inputs["b3"])
    return (X_hat, W), res


def kernel(**inputs):
    (X_hat, W), _ = run(trace=False, **inputs)
    return (X_hat, W)


# revision 30
# speedup vs baseline: 1.0402x; 1.0179x over previous
"""Trainium2 Bass kernel for NeuralCausalDiscovery (28 grouped MLP mechanisms,
BatchNorm (training stats) + exact GELU, batch 32768).

Sharding: expert-parallel over the 28 independent mechanisms. 8 cores x 4
mechanism slots (cores 4-7 carry one duplicated pad slot). Each core processes
the FULL batch for its mechanisms, so BN batch statistics are exact with no
collectives.

Per-core dataflow (all activations in [feature, batch] layout, 2 groups of 2
mechanisms = 128 partitions):
  - Gram: G_aug = [X|1]^T [X|1] accumulated on PE (4 concurrent col-tiles).
    BN1 stats derive algebraically from G (linear-layer + Gram identity), so
    layer-1 BN+GELU is a single fused ACT op straight out of PSUM.
  - L1: h1 = W1eff @ X^T (fp16 matmuls), gelu1 = Gelu(s1*h1+b1adj) PSUM->SBUF.
  - L2 pass A: h2 streamed to PSUM, bn_stats consumes it (stats only).
  - L2 pass B: h2 recomputed, gelu2 fused with BN2 at PSUM read.
  - L3: out = W3 @ a2, [2,512] PSUM tiles DMA'd straight to DRAM.
"""

import sys

import numpy as np

if "/opt/trn_rl_repo" not in sys.path:
    sys.path.insert(0, "/opt/trn_rl_repo")

N_VARS, HID, BATCH = 28, 64, 32768
EPS = 1e-5
SUB = 512           # matmul N / bn_stats subgroup width
CHUNK = 1024        # psum chunk columns (2 banks)
NSUB_TOT = BATCH // SUB
N_CORES = 8

_CACHE = {}


def _assignments():
    """Per-core list of 4 mechanism slots (cores 4-7 pad with a duplicate)."""
    cores = []
    idx = 0
    for c in range(N_CORES):
        k = 4 if c < 4 else 3
        mechs = list(range(idx, idx + k))
        idx += k
        while len(mechs) < 4:
            mechs.append(mechs[0])
        cores.append(mechs)
    assert idx == N_VARS
    return cores


def _chunks():
    out = []
    c0 = 0
    while c0 < BATCH:
        ln = min(CHUNK, BATCH - c0)
        out.append((c0, ln))
        c0 += ln
    return out


def _build_module():
    import concourse.bass as bass
    import concourse.tile as tile
    from concourse import mybir

    dt = mybir.dt
    f32, f16 = dt.float32, dt.float16
    AF = mybir.ActivationFunctionType
    AX = mybir.AxisListType
    ALU = mybir.AluOpType
    B = BATCH

    nc = bass.Bass(target_bir_lowering=False)

    Xaug = nc.declare_dram_parameter("Xaug", [B, 29], f16, isOutput=False).ap()
    XT16 = nc.declare_dram_parameter("XT16", [28, B], f16, isOutput=False).ap()
    W116 = nc.declare_dram_parameter("W116", [2, 28, 128], f16, isOutput=False).ap()
    W132 = nc.declare_dram_parameter("W132", [2, 28, 128], f32, isOutput=False).ap()
    W1T32 = nc.declare_dram_parameter("W1T32", [2, 128, 28], f32, isOutput=False).ap()
    W216 = nc.declare_dram_parameter("W216", [2, 128, 128], f16, isOutput=False).ap()
    W316 = nc.declare_dram_parameter("W316", [2, 128, 32], f16, isOutput=False).ap()
    GB = nc.declare_dram_parameter("GB", [128, 8], f32, isOutput=False).ap()
    OUT = nc.declare_dram_parameter("OUT", [4, B], f32, isOutput=True).ap()

    Xr4 = Xaug.rearrange("(n c p) d -> n c p d", c=4, p=128)  # [64, 4, 128, 29]

    with tile.TileContext(nc) as tc:
        with (
            tc.tile_pool(name="big", bufs=2) as big,          # a1 buffers
            tc.tile_pool(name="xa", bufs=4) as xap,           # gram input chunks
            tc.tile_pool(name="xt", bufs=5) as xtp,           # XT streaming chunks
            tc.tile_pool(name="a2", bufs=3) as a2p,           # gelu2 output ring
            tc.tile_pool(name="wts", bufs=1) as wts,          # weights + vectors
            tc.tile_pool(name="hps", bufs=3, space="PSUM") as hps,    # h psum chunks
            tc.tile_pool(name="bps", bufs=2, space="PSUM") as bps,    # B/L3/gram/stats psum
        ):
            # ---------------- weight / constant loads ----------------
            epsT = wts.tile([128, 1], f32, tag="eps")
            nc.vector.memset(epsT, EPS)
            gb = wts.tile([128, 8], f32, tag="gb")
            nc.gpsimd.dma_start(out=gb, in_=GB)
            w116 = []
            w132 = []
            w1T32 = []
            w216 = []
            w316 = []
            for g in range(2):
                t = wts.tile([28, 128], f16, tag=f"w116_{g}")
                nc.gpsimd.dma_start(out=t, in_=W116[g])
                w116.append(t)
                t = wts.tile([28, 128], f32, tag=f"w132_{g}")
                nc.gpsimd.dma_start(out=t, in_=W132[g])
                w132.append(t)
                t = wts.tile([128, 28], f32, tag=f"w1T32_{g}")
                nc.gpsimd.dma_start(out=t, in_=W1T32[g])
                w1T32.append(t)
                t = wts.tile([128, 128], f16, tag=f"w216_{g}")
                nc.gpsimd.dma_start(out=t, in_=W216[g])
                w216.append(t)
                t = wts.tile([128, 32], f16, tag=f"w316_{g}")
                nc.gpsimd.dma_start(out=t, in_=W316[g])
                w316.append(t)

            # ---------------- phase 0: Gram of [X|1] ----------------
            # 4 concurrent accumulation chains on PE col-groups 0..3.
            g_ps = bps.tile([128, SUB], f32, tag="hb", name="g_ps")
            for half in range(8):
                xa = xap.tile([128, 32, 29], f16, tag="xa")
                eng = nc.sync if half % 2 == 0 else nc.gpsimd
                eng.dma_start(
                    out=xa, in_=Xr4[8 * half : 8 * (half + 1)]
                    .rearrange("n c p d -> p (n c) d")
                )
                for jj in range(32):
                    t = 32 * half + jj
                    j = t % 4
                    nc.tensor.matmul(
                        out=g_ps[32 * j : 32 * j + 28, 0:29],
                        lhsT=xa[:, jj, 0:28],
                        rhs=xa[:, jj, 0:29],
                        start=(t < 4),
                        stop=(t >= 252),
                        tile_position=(0, 32 * j),
                        skip_group_check=True,
                    )
            # reduce the 4 partial grams -> g_sb [28, 29]
            g_sb = wts.tile([28, 29], f32, tag="g_sb")
            nc.vector.tensor_copy(out=g_sb, in_=g_ps[0:28, 0:29])
            for j in range(1, 4):
                nc.vector.tensor_add(
                    out=g_sb, in0=g_sb, in1=g_ps[32 * j : 32 * j + 28, 0:29]
                )

            # ---------------- per-group BN1 stats from Gram ----------------
            s1v, b1v = [], []
            for g in range(2):
                cs = wts.tile([28, 1], f32, tag=f"cs_{g}")
                nc.vector.tensor_scalar_mul(cs, g_sb[:, 28:29], 1.0 / B)
                mean_ps = bps.tile([128, SUB], f32, tag="hb", name="mean_ps")
                nc.tensor.matmul(out=mean_ps[:, 0:1], lhsT=w132[g], rhs=cs)
                m1 = wts.tile([128, 1], f32, tag=f"m1_{g}")
                nc.vector.tensor_copy(out=m1, in_=mean_ps[:, 0:1])

                t2_ps = bps.tile([128, SUB], f32, tag="hb", name="t2_ps")
                nc.tensor.matmul(out=t2_ps[:, 0:28], lhsT=w132[g], rhs=g_sb[:, 0:28])
                tmp = wts.tile([128, 28], f32, tag=f"tmp28_{g}")
                nc.vector.tensor_mul(out=tmp, in0=t2_ps[:, 0:28], in1=w1T32[g])
                e2 = wts.tile([128, 1], f32, tag=f"e2_{g}")
                nc.vector.tensor_reduce(
                    out=e2, in_=tmp, axis=AX.X, op=ALU.add
                )
                nc.vector.tensor_scalar_mul(e2, e2, 1.0 / B)
                m1sq = wts.tile([128, 1], f32, tag=f"m1sq_{g}")
                nc.vector.tensor_mul(out=m1sq, in0=m1, in1=m1)
                var1 = wts.tile([128, 1], f32, tag=f"var1_{g}")
                nc.vector.tensor_sub(out=var1, in0=e2, in1=m1sq)
                sd1 = wts.tile([128, 1], f32, tag=f"sd1_{g}")
                nc.scalar.activation(sd1, var1, AF.Sqrt, bias=epsT, scale=1.0)
                rinv = wts.tile([128, 1], f32, tag=f"rinv_{g}")
                nc.vector.reciprocal(rinv, sd1)
                s1 = wts.tile([128, 1], f32, tag=f"s1_{g}")
                nc.vector.tensor_mul(out=s1, in0=rinv, in1=gb[:, 4 * g : 4 * g + 1])
                tb = wts.tile([128, 1], f32, tag=f"tb_{g}")
                nc.vector.tensor_mul(out=tb, in0=s1, in1=m1)
                b1 = wts.tile([128, 1], f32, tag=f"b1_{g}")
                nc.vector.tensor_sub(
                    out=b1, in0=gb[:, 4 * g + 1 : 4 * g + 2], in1=tb
                )
                s1v.append(s1)
                b1v.append(b1)

            chunks = _chunks()

            a1t = [
                big.tile([128, B], f16, tag="a1", name="a1_g0"),
                big.tile([128, B], f16, tag="a1", name="a1_g1"),
            ]

            def phase_A(g):
                """L1 + fused BN1-GELU -> a1[g]."""
                XTCH = 2048
                xts = {}

                def load_xt(x0):
                    if x0 not in xts and x0 < B:
                        xt = xtp.tile([28, XTCH], f16, tag="xt", name="xt")
                        nc.gpsimd.dma_start(out=xt, in_=XT16[:, x0 : x0 + XTCH])
                        xts[x0] = xt

                load_xt(0)
                load_xt(XTCH)
                for (c0, ln) in chunks:
                    hp = hps.tile([128, CHUNK], f32, tag="hp")
                    for s in range(ln // SUB):
                        cc = c0 + s * SUB
                        x0 = (cc // XTCH) * XTCH
                        load_xt(x0 + XTCH)
                        xt = xts[x0]
                        xo = cc % XTCH
                        nc.tensor.matmul(
                            out=hp[:, s * SUB : (s + 1) * SUB],
                            lhsT=w116[g],
                            rhs=xt[:, xo : xo + SUB],
                        )
                    nc.scalar.activation(
                        a1t[g][:, c0 : c0 + ln],
                        hp[:, 0:ln],
                        AF.Gelu,
                        bias=b1v[g],
                        scale=s1v[g],
                    )

            def phase_B(g, st):
                """L2 stats pass: h2 -> PSUM -> bn_stats, h2 discarded."""
                for sub in range(NSUB_TOT):
                    hp = bps.tile([128, SUB], f32, tag="hb", name="hb")
                    nc.tensor.matmul(
                        out=hp,
                        lhsT=w216[g],
                        rhs=a1t[g][:, sub * SUB : (sub + 1) * SUB],
                    )
                    nc.vector.bn_stats(out=st[:, sub, :], in_=hp)

            def stats2(g, st):
                mv = wts.tile([128, 2], f32, tag=f"mv_{g}")
                nc.vector.bn_aggr(out=mv, in_=st)
                sd2 = wts.tile([128, 1], f32, tag=f"sd2_{g}")
                nc.scalar.activation(sd2, mv[:, 1:2], AF.Sqrt, bias=epsT, scale=1.0)
                rinv2 = wts.tile([128, 1], f32, tag=f"rinv2_{g}")
                nc.vector.reciprocal(rinv2, sd2)
                s2 = wts.tile([128, 1], f32, tag=f"s2_{g}")
                nc.vector.tensor_mul(
                    out=s2, in0=rinv2, in1=gb[:, 4 * g + 2 : 4 * g + 3]
                )
                tb2 = wts.tile([128, 1], f32, tag=f"tb2_{g}")
                nc.vector.tensor_mul(out=tb2, in0=s2, in1=mv[:, 0:1])
                b2 = wts.tile([128, 1], f32, tag=f"b2_{g}")
                nc.vector.tensor_sub(
                    out=b2, in0=gb[:, 4 * g + 3 : 4 * g + 4], in1=tb2
                )
                return s2, b2

            def phase_C(g, s2, b2):
                """L2 recompute + fused BN2-GELU + L3 (4 sub-chunks packed per
                PSUM bank at partition offsets 0/32/64/96) + copy + DMA out."""
                sub_global = 0
                op = None
                pend = []
                for (c0, ln) in chunks:
                    hp = hps.tile([128, CHUNK], f32, tag="hp")
                    ns = ln // SUB
                    for s in range(ns):
                        nc.tensor.matmul(
                            out=hp[:, s * SUB : (s + 1) * SUB],
                            lhsT=w216[g],
                            rhs=a1t[g][:, c0 + s * SUB : c0 + (s + 1) * SUB],
                        )
                    a2 = a2p.tile([128, CHUNK], f16, tag="a2")
                    nc.scalar.activation(
                        a2[:, 0:ln], hp[:, 0:ln], AF.Gelu, bias=b2, scale=s2
                    )
                    for s in range(ns):
                        q = sub_global % 4
                        j = (sub_global // 4) % 4
                        if q == 0:
                            op = bps.tile([128, SUB], f32, tag="hb", name="op")
                        if sub_global % 16 == 0:
                            stg = a2p.tile([128, 4 * SUB], f32, tag="stg",
                                           name="stg")
                            base8k = c0 + s * SUB
                        nc.tensor.matmul(
                            out=op[32 * q : 32 * q + 32, :],
                            lhsT=w316[g],
                            rhs=a2[:, s * SUB : (s + 1) * SUB],
                            tile_position=(0, 32 * q),
                            skip_group_check=True,
                        )
                        if q == 3:
                            nc.vector.tensor_copy(
                                out=stg[:, j * SUB : (j + 1) * SUB], in_=op
                            )
                        if sub_global % 16 == 15:
                            row16 = OUT[2 * g : 2 * g + 2,
                                        base8k : base8k + 16 * SUB]
                            for qq in range(4):
                                for r in range(2):
                                    out_ap = row16[r : r + 1].rearrange(
                                        "a (jj q f) -> a jj q f", q=4, f=SUB
                                    )[:, :, qq, :]
                                    in_ap = stg[
                                        32 * qq + r : 32 * qq + r + 1, :
                                    ].rearrange("a (jj f) -> a jj f", f=SUB)
                                    nc.sync.dma_start(out=out_ap, in_=in_ap)
                        sub_global += 1

            st0 = wts.tile([128, NSUB_TOT, 6], f32, tag="st_0")
            st1 = wts.tile([128, NSUB_TOT, 6], f32, tag="st_1")

            # pipelined schedule: B(g) overlaps A(g+1); B(1) overlaps C(0)
            phase_A(0)
            phase_B(0, st0)
            s2_0, b2_0 = stats2(0, st0)
            phase_A(1)
            phase_B(1, st1)
            phase_C(0, s2_0, b2_0)
            s2_1, b2_1 = stats2(1, st1)
            phase_C(1, s2_1, b2_1)

    _split_multi_waits(nc, mybir)
    return nc


def _split_multi_waits(nc, mybir):
    """This walrus build accepts only one sync-wait per instruction; hoist
    extra waits onto standalone NoOps on the same engine stream."""
    for fn in nc.m.functions:
        for blk in fn.blocks:
            insts = list(blk.instructions)
            out = []
            nsplit = 0
            for inst in insts:
                si = inst.sync_info
                if si is not None and si.on_wait and len(si.on_wait) > 1:
                    waits = list(si.on_wait)
                    for w in waits[:-1]:
                        nop = mybir.InstNoOp(
                            name=nc.get_next_instruction_name(),
                            engine=inst.engine,
                            ins=[],
                            outs=[],
                            sync_info=mybir.SyncInfo(on_wait=[w], on_update=[]),
                            bass_nofuse=True,
                        )
                        nc.register_instruction(nop)
                        out.append(nop)
                        nsplit += 1
                    inst.sync_info = mybir.SyncInfo(
                        on_wait=[waits[-1]], on_update=list(si.on_update)
                    )
                out.append(inst)
            if nsplit:
                li = blk.instructions
                li.clear()
                for i in out:
                    blk.add_instruction(i)


def _get_nc():
    if "nc" not in _CACHE:
        _CACHE["nc"] = _build_module()
    return _CACHE["nc"]


def _host_prep(X, W_logits, W1, b1, gamma1, beta1, W2, b2, gamma2, beta2, W3, b3):
    """Build per-core input maps. Weight-only preprocessing on host."""
    X = np.asarray(X, np.float32)
    W_logits = np.asarray(W_logits, np.float32)
    W1 = np.asarray(W1, np.float32)
    W2 = np.asarray(W2, np.float32)
    W3 = np.asarray(W3, np.float32)
    gamma1 = np.asarray(gamma1, np.float32)
    beta1 = np.asarray(beta1, np.float32)
    gamma2 = np.asarray(gamma2, np.float32)
    beta2 = np.asarray(beta2, np.float32)

    W = (1.0 / (1.0 + np.exp(-W_logits))) * (1.0 - np.eye(N_VARS, dtype=np.float32))
    W1eff = W1 * W[:, None, :]  # [n, h, n]

    Xaug = np.ones((BATCH, 29), np.float16)
    Xaug[:, 0:28] = X.astype(np.float16)
    XT16 = np.ascontiguousarray(X.T).astype(np.float16)

    assigns = _assignments()
    in_maps = []
    for c in range(N_CORES):
        mechs = assigns[c]
        W1g = np.zeros((2, 28, 128), np.float32)
        W2g = np.zeros((2, 128, 128), np.float32)
        W3g = np.zeros((2, 128, 2), np.float32)
        GBg = np.zeros((128, 8), np.float32)
        for g in range(2):
            m0, m1 = mechs[2 * g], mechs[2 * g + 1]
            W1g[g, :, 0:64] = W1eff[m0].T
            W1g[g, :, 64:128] = W1eff[m1].T
            W2g[g, 0:64, 0:64] = W2[m0].T
            W2g[g, 64:128, 64:128] = W2[m1].T
            W3g[g, 0:64, 0] = W3[m0, 0, :]
            W3g[g, 64:128, 1] = W3[m1, 0, :]
            GBg[:, 4 * g + 0] = np.concatenate([gamma1[m0], gamma1[m1]])
            GBg[:, 4 * g + 1] = np.concatenate([beta1[m0], beta1[m1]])
            GBg[:, 4 * g + 2] = np.concatenate([gamma2[m0], gamma2[m1]])
            GBg[:, 4 * g + 3] = np.concatenate([beta2[m0], beta2[m1]])
        W1gT = np.ascontiguousarray(np.transpose(W1g, (0, 2, 1)))
        in_maps.append(
            {
                "Xaug": Xaug,
                "XT16": XT16,
                "W116": W1g.astype(np.float16),
                "W132": W1g,
                "W1T32": W1gT,
                "W216": W2g.astype(np.float16),
                "W316": np.tile(W3g, (1, 1, 16)).astype(np.float16),
                "GB": GBg,
            }
        )
    return in_maps, assigns, W


def _assemble(results, assigns, W, b3):
    b3 = np.asarray(b3, np.float32)
    X_hat = np.zeros((BATCH, N_VARS), np.float32)
    done = set()
    for c in range(N_CORES):
        out = results[c]["OUT"]  # [4, B]
        for slot in range(4):
            mech = assigns[c][slot]
            if mech in done:
                continue
            done.add(mech)
            X_hat[:, mech] = out[slot, :] + b3[mech, 0]
    assert len(done) == N_VARS
    return X_hat, W


def run(trace=False, **inputs):
    from concourse.bass_utils import run_bass_kernel_spmd

    in_maps, assigns, W = _host_prep(**inputs)
    nc = _get_nc()
    res = run_bass_kernel_spmd(nc, in_maps, list(range(N_CORES)), trace=trace)
    X_hat, W = _assemble(res.results, assigns, W, inputs["b3"])
    return (X_hat, W), res


def kernel(**inputs):
    (X_hat, W), _ = run(trace=False, **inputs)
    return (X_hat, W)
